# revision 1
# baseline (speedup 1.0000x reference)
"""Trainium2 Bass kernel for nn_BaselineDNN (embedding pooling + MLP).

Reference computation (B=2048, L=200, V=50000, D=300, H=128, C=20):
    emb = emb_table[x]                       # [B, L, D] gather
    s   = sum(emb, axis=1); mx = max(emb, axis=1)
    rep = concat([s / len^2, mx], -1)        # [B, 600]
    h   = relu(rep @ W_new.T + b_new)        # [B, 128]
    out = h @ W3.T + b3                      # [B, 20]

Sharding: data-parallel over batch across 8 cores (256 rows/core),
emb table + weights replicated. No collectives.

Per-core device program (layout: batch row on partitions, 2 groups of 128):
  - indirect-DMA gather of emb rows in token chunks -> SBUF [128, csz, 300]
  - max: DVE tensor_reduce over the (strided) token axis
  - sum: PE identity-matmul accumulation into PSUM
  - mean_bug scale, rep assembly, PE transpose of rep, 2-layer MLP on PE/ACT
"""

import numpy as np

import concourse.bacc as bacc
import concourse.bass as bass
import concourse.mybir as mybir
import concourse.tile as tile
from concourse.bass_utils import run_bass_kernel_spmd

F32 = mybir.dt.float32
I32 = mybir.dt.int32

B, L, V, D, H, C = 2048, 200, 50000, 300, 128, 20
NCORES = 8
BL = B // NCORES          # 256 rows per core
P = 128                   # partitions
G = BL // P               # 2 groups of 128 rows
KD = 5                    # d-chunks of 128 for the 600-dim rep (640 padded)
DPAD = KD * P             # 640
# token chunks per group (sum = L)
CHUNK = 32
CHUNKS = [CHUNK] * (L // CHUNK) + ([L % CHUNK] if L % CHUNK else [])


def build_program(gather_bufs: int = 3, nq: int = 1):
    nc = bacc.Bacc(
        "TRN2", target_bir_lowering=False, debug=False, num_swdge_queues=nq
    )

    emb = nc.dram_tensor("emb", [V, D], F32, kind="ExternalInput").ap()
    idx = nc.dram_tensor("idx", [P, G * L], I32, kind="ExternalInput").ap()
    invl = nc.dram_tensor("invl", [P, G], F32, kind="ExternalInput").ap()
    wnewt = nc.dram_tensor("wnewt", [KD, P, H], F32, kind="ExternalInput").ap()
    w3t = nc.dram_tensor("w3t", [H, C], F32, kind="ExternalInput").ap()
    bnew = nc.dram_tensor("bnew", [H, 1], F32, kind="ExternalInput").ap()
    b3 = nc.dram_tensor("b3", [C, 1], F32, kind="ExternalInput").ap()
    iden = nc.dram_tensor("iden", [P, P], F32, kind="ExternalInput").ap()
    out = nc.dram_tensor("out", [C, BL], F32, kind="ExternalOutput").ap()

    with tile.TileContext(nc) as tc:
        with (
            tc.tile_pool(name="const", bufs=1) as const_pool,
            tc.tile_pool(name="gath", bufs=gather_bufs) as gather_pool,
            tc.tile_pool(name="work", bufs=2) as work_pool,
            tc.tile_pool(name="psum", bufs=2, space="PSUM") as psum_pool,
        ):
            idx_sb = const_pool.tile([P, G * L], I32)
            nc.sync.dma_start(out=idx_sb[:], in_=idx[:])
            invl_sb = const_pool.tile([P, G], F32)
            nc.sync.dma_start(out=invl_sb[:], in_=invl[:])
            iden_sb = const_pool.tile([P, P], F32)
            nc.sync.dma_start(out=iden_sb[:], in_=iden[:])
            # single DMA (one completion sem) via transposed dram view
            wnewt_sb = const_pool.tile([P, KD, H], F32)
            nc.sync.dma_start(out=wnewt_sb[:], in_=wnewt[:].transpose([1, 0, 2]))
            w3t_sb = const_pool.tile([H, C], F32)
            nc.sync.dma_start(out=w3t_sb[:], in_=w3t[:])
            bnew_sb = const_pool.tile([H, 1], F32)
            nc.sync.dma_start(out=bnew_sb[:], in_=bnew[:])
            b3_sb = const_pool.tile([C, 1], F32)
            nc.sync.dma_start(out=b3_sb[:], in_=b3[:])

            # [d-part, k-chunk, batch(2 groups)] transposed rep for the MLP
            rep_t = const_pool.tile([P, KD, BL], F32)


            # history of (gather tile, partials slice) per global chunk, for
            # the wait-absorbing touches B chunks later
            hist = []
            for g in range(G):
                psum_s = psum_pool.tile([P, D], F32, tag="psum_s")
                partials = work_pool.tile([P, len(CHUNKS), D], F32, tag="partials")
                c0 = 0
                for ci, csz in enumerate(CHUNKS):
                    gi = len(hist)
                    gt = gather_pool.tile([P, CHUNK, D], F32, tag="gt")
                    # one index per partition per DMA — the only offset shape
                    # the HW SWDGE indirect1d path supports (multi-column
                    # offsets land permuted/partial on real silicon)
                    for j in range(csz):
                        col = g * L + c0 + j
                        ginst = nc.gpsimd.indirect_dma_start(
                            out=gt[:, j, :],
                            out_offset=None,
                            in_=emb[:],
                            in_offset=bass.IndirectOffsetOnAxis(
                                ap=idx_sb[:, col : col + 1],
                                axis=0,
                            ),
                        )
                        if nq > 1:
                            ginst.ins.queue = f"qPoolDynamic{(col % nq) or ''}"
                    hist.append((gt, partials[0:1, ci, 0:1]))
                    # running max over this chunk's tokens (strided axis)
                    nc.vector.tensor_reduce(
                        out=partials[:, ci, :],
                        in_=gt[:, :csz, :].transpose([0, 2, 1]),
                        axis=mybir.AxisListType.X,
                        op=mybir.AluOpType.max,
                    )
                    # sum: accumulate each token column into PSUM via identity matmul
                    for j in range(csz):
                        nc.tensor.matmul(
                            out=psum_s[:],
                            lhsT=iden_sb[:],
                            rhs=gt[:, j, :],
                            start=(c0 + j == 0),
                            stop=(c0 + j == L - 1),
                        )
                    c0 += csz

                rep = work_pool.tile([P, DPAD], F32, tag="rep")
                nc.vector.memset(rep[:, 2 * D : DPAD], 0.0)
                # mean_bug = s / len^2
                nc.vector.tensor_scalar_mul(rep[:, 0:D], psum_s[:], invl_sb[:, g : g + 1])
                # final max across chunk partials
                nc.vector.tensor_reduce(
                    out=rep[:, D : 2 * D],
                    in_=partials[:].transpose([0, 2, 1]),
                    axis=mybir.AxisListType.X,
                    op=mybir.AluOpType.max,
                )
                # transpose rep -> rep_t[:, k, g*128:(g+1)*128]
                for k in range(KD):
                    pt = psum_pool.tile([P, P], F32, tag="pt")
                    nc.tensor.transpose(
                        out=pt[:],
                        in_=rep[:, k * P : (k + 1) * P],
                        identity=iden_sb[:],
                    )
                    nc.vector.tensor_copy(out=rep_t[:, k, g * P : (g + 1) * P], in_=pt[:])

            # h = relu(rep @ W_new.T + b_new): out[h, b]
            psum_h = psum_pool.tile([P, BL], F32, tag="psum_h", bufs=1)
            for k in range(KD):
                nc.tensor.matmul(
                    out=psum_h[:],
                    lhsT=wnewt_sb[:, k, :],
                    rhs=rep_t[:, k, :],
                    start=(k == 0),
                    stop=(k == KD - 1),
                )
            h_sb = work_pool.tile([P, BL], F32)
            nc.scalar.activation(
                h_sb[:],
                psum_h[:],
                mybir.ActivationFunctionType.Relu,
                bias=bnew_sb[:],
                scale=1.0,
            )
            # logits = h @ W3.T + b3: out[c, b]
            psum_l = psum_pool.tile([C, BL], F32, tag="psum_l", bufs=1)
            nc.tensor.matmul(
                out=psum_l[:], lhsT=w3t_sb[:], rhs=h_sb[:], start=True, stop=True
            )
            lo_sb = work_pool.tile([C, BL], F32)
            nc.vector.tensor_scalar_add(lo_sb[:], psum_l[:], b3_sb[:])
            nc.sync.dma_start(out=out[:], in_=lo_sb[:])

    nc.compile()
    return nc


def make_in_maps(x, lengths, emb_table, W_new, b_new, W3, b3):
    emb_np = np.ascontiguousarray(emb_table, dtype=np.float32)
    x_np = np.asarray(x).astype(np.int32)
    len_f = np.asarray(lengths).astype(np.float32)
    inv_len2 = (1.0 / (len_f * len_f)).astype(np.float32)

    wnewt_pad = np.zeros((DPAD, H), dtype=np.float32)
    wnewt_pad[: 2 * D, :] = np.asarray(W_new, dtype=np.float32).T
    wnewt_np = np.ascontiguousarray(wnewt_pad.reshape(KD, P, H))
    w3t_np = np.ascontiguousarray(np.asarray(W3, dtype=np.float32).T)
    bnew_np = np.asarray(b_new, dtype=np.float32).reshape(H, 1)
    b3_np = np.asarray(b3, dtype=np.float32).reshape(C, 1)
    iden_np = np.eye(P, dtype=np.float32)

    in_maps = []
    for c in range(NCORES):
        xl = x_np[c * BL : (c + 1) * BL]            # [256, 200]
        il = inv_len2[c * BL : (c + 1) * BL]        # [256]
        idx_np = np.ascontiguousarray(
            xl.reshape(G, P, L).transpose(1, 0, 2).reshape(P, G * L)
        )
        invl_np = np.ascontiguousarray(il.reshape(G, P).T)
        in_maps.append(
            {
                "emb": emb_np,
                "idx": idx_np,
                "invl": invl_np,
                "wnewt": wnewt_np,
                "w3t": w3t_np,
                "bnew": bnew_np,
                "b3": b3_np,
                "iden": iden_np,
            }
        )
    return in_maps


def run(inputs, trace=False, gather_bufs=3, tmpdir=None, nq=1):
    nc = build_program(gather_bufs=gather_bufs, nq=nq)
    in_maps = make_in_maps(**inputs)
    res = run_bass_kernel_spmd(
        nc, in_maps, core_ids=list(range(NCORES)), trace=trace, tmpdir=tmpdir
    )
    outs = [res.results[c]["out"].T for c in range(NCORES)]  # each [256, 20]
    full = np.concatenate(outs, axis=0).astype(np.float32)
    return full, res


def kernel(**inputs) -> np.ndarray:
    full, _ = run(inputs, trace=False)
    return full

